# revision 1
# baseline (speedup 1.0000x reference)
"""Trainium2 Bass kernel for 3D neighborhood attention (sparse_attention).

Problem: q,k [1,40,40,40,48] fp32, rpb [8,3,3,3]; out [1,24,40,40,40].
Per voxel x: logits[h,kk] = scale * <q[x,h,:], k[x+off_kk,h,:]> + rpb[h,kk]
(zero-padded k at boundaries, kk over 3x3x3 offsets), p = softmax over kk,
out[x,h,:] = sum_kk p[h,kk] * off_kk  (constant integer offsets as values).

Sharding: spatial-parallel over H (40 -> 8 slabs of 5). Each core gets its
q slab plus a host-side im2col of the 27 shifted k views for its slab
(halo handled on host), so on-core everything is token-parallel with
tokens on SBUF partitions (2 tokens per partition) and no cross-partition
data movement. The PV contraction exploits that the "values" are the
constant offsets in {-1,0,1}^3: out_i = (sum of exp over di=+1 block) -
(sum over di=-1 block), so it is pure block reductions, no multiplies.
"""

import numpy as np

import concourse.bass as bass
import concourse.tile as tile
from concourse import bacc, mybir
from concourse.bass_utils import run_bass_kernel_spmd

NH = 8
HD = 6
DIM = NH * HD
KS = 3
NT = KS**3  # 27
SCALE = HD**-0.5
H = W = T = 40
N_CORES = 8
SLAB = H // N_CORES          # 5 rows of H per core
TOK = SLAB * W * T           # 8000 tokens per core
P = 128
TPP = 2                      # tokens per partition
TILES = 32                   # ceil(8000 / 256)
TOKP = TILES * P * TPP       # 8192
FKC = NT * DIM               # 1296  (kk, c) free dim per token
FKH = NT * NH                # 216   (kk, h) free dim per token

_prog_cache = {}


def _build_program():
    fp32 = mybir.dt.float32
    nc = bacc.Bacc("TRN2", target_bir_lowering=False, debug=False,
                   num_devices=N_CORES)
    qs = nc.dram_tensor("qs", [TILES, P, TPP * DIM], fp32,
                        kind="ExternalInput").ap()
    kn = nc.dram_tensor("kn", [TILES, P, TPP * FKC], fp32,
                        kind="ExternalInput").ap()
    rpbt = nc.dram_tensor("rpbt", [P, FKH], fp32, kind="ExternalInput").ap()
    out = nc.dram_tensor("out", [TILES, P, TPP * 3 * NH], fp32,
                         kind="ExternalOutput").ap()

    X = mybir.AxisListType.X
    XY = mybir.AxisListType.XY
    ADD = mybir.AluOpType.add

    with tile.TileContext(nc) as tc:
        with (
            tc.tile_pool(name="consts", bufs=1) as cpool,
            tc.tile_pool(name="kin", bufs=3) as kpool,
            tc.tile_pool(name="qin", bufs=3) as qpool,
            tc.tile_pool(name="prod", bufs=2) as ppool,
            tc.tile_pool(name="logit", bufs=3) as lpool,
            tc.tile_pool(name="expv", bufs=3) as epool,
            tc.tile_pool(name="small", bufs=16) as spool,
            tc.tile_pool(name="outp", bufs=3) as opool,
        ):
            rpb_sb = cpool.tile([P, FKH], fp32)
            nc.sync.dma_start(rpb_sb[:], rpbt[:])

            for ti in range(TILES):
                kt = kpool.tile([P, TPP * FKC], fp32)
                nc.sync.dma_start(kt[:], kn[ti])
                qt = qpool.tile([P, TPP * DIM], fp32)
                nc.sync.dma_start(qt[:], qs[ti])

                # P4[p, j, kk, c] = kn[p, j, kk, c] * q[p, j, c]
                pt = ppool.tile([P, TPP * FKC], fp32)
                q_b = (qt[:].rearrange("p (j c) -> p j c", j=TPP)
                       .unsqueeze(2).broadcast_to([P, TPP, NT, DIM]))
                nc.vector.tensor_mul(
                    pt[:].rearrange("p (j kk c) -> p j kk c", j=TPP, kk=NT),
                    kt[:].rearrange("p (j kk c) -> p j kk c", j=TPP, kk=NT),
                    q_b,
                )
                # L[p, (j,kk,h)] = sum_d P4[p, j, (kk,h), d]
                lt = lpool.tile([P, TPP * FKH], fp32)
                nc.vector.tensor_reduce(
                    lt[:],
                    pt[:].rearrange("p (j kh d) -> p j kh d", j=TPP, d=HD),
                    axis=X, op=ADD,
                )
                # L2 = L + rpb  (q was pre-scaled by SCALE on host)
                l2 = lpool.tile([P, TPP * FKH], fp32)
                rpb_b = rpb_sb[:].unsqueeze(1).broadcast_to([P, TPP, FKH])
                nc.vector.tensor_add(
                    l2[:].rearrange("p (j f) -> p j f", j=TPP),
                    lt[:].rearrange("p (j f) -> p j f", j=TPP),
                    rpb_b,
                )
                # E = exp(L2)  (ScalarE, overlaps with DVE)
                et = epool.tile([P, TPP * FKH], fp32)
                nc.scalar.activation(et[:], l2[:],
                                     mybir.ActivationFunctionType.Exp)

                # Softmax denominator: S0[p, (j,h)] = sum_kk E
                e_khk = et[:].rearrange("p (j kk h) -> p j h kk",
                                        j=TPP, kk=NT, h=NH)
                s0 = spool.tile([P, TPP * NH], fp32)
                nc.vector.tensor_reduce(s0[:], e_khk, axis=X, op=ADD)

                # Directional numerators via paired block sums over the
                # +-1 slabs of each axis (values are +-1/0).
                # E free layout: (j, di, dj, dl, h).  V layout: (o, j, pm, h)
                v_di = et[:].rearrange(
                    "p (j di dj dl h) -> p j di h (dj dl)",
                    j=TPP, di=KS, dj=KS, dl=KS, h=NH)
                v_dj = et[:].rearrange(
                    "p (j di dj dl h) -> p j dj h di dl",
                    j=TPP, di=KS, dj=KS, dl=KS, h=NH)
                v_dl = et[:].rearrange(
                    "p (j di dj dl h) -> p j dl h di dj",
                    j=TPP, di=KS, dj=KS, dl=KS, h=NH)

                vt = spool.tile([P, 3 * 2 * TPP * NH], fp32)  # [128, 96]
                npm = TPP * NH
                for o, (v, ax) in enumerate(((v_di, X), (v_dj, XY),
                                             (v_dl, XY))):
                    for pm in range(2):
                        nc.vector.tensor_reduce(
                            vt[:, (o * 2 + pm) * npm:(o * 2 + pm + 1) * npm],
                            v[:, :, 2 * pm], axis=ax, op=ADD)

                # S3[p, (o,j,h)] = V[.., pm=1] - V[.., pm=0]
                v5 = vt[:].rearrange("p (o pm j h) -> p o pm j h",
                                     o=3, pm=2, j=TPP)
                s3 = spool.tile([P, 3 * TPP * NH], fp32)
                nc.vector.tensor_sub(
                    s3[:].rearrange("p (o j h) -> p o j h", o=3, j=TPP),
                    v5[:, :, 1], v5[:, :, 0])

                rt = spool.tile([P, TPP * NH], fp32)
                nc.vector.reciprocal(rt[:], s0[:])
                # out[p, (o,j,h)] = S3 * (1/S0)
                ot = opool.tile([P, TPP * 3 * NH], fp32)
                r_b = (rt[:].rearrange("p (j h) -> p j h", j=TPP)
                       .unsqueeze(1).broadcast_to([P, 3, TPP, NH]))
                nc.vector.tensor_mul(
                    ot[:].rearrange("p (o j h) -> p o j h", o=3, j=TPP),
                    s3[:].rearrange("p (o j h) -> p o j h", o=3, j=TPP),
                    r_b)
                nc.sync.dma_start(out[ti], ot[:])

    nc.compile()
    return nc


def _host_prep(q, k, rpb):
    q = np.asarray(q, dtype=np.float32)
    k = np.asarray(k, dtype=np.float32)
    rpb = np.asarray(rpb, dtype=np.float32)

    q0 = (q[0] * SCALE).astype(np.float32)          # [40,40,40,48]
    kp = np.pad(k[0], ((1, 1), (1, 1), (1, 1), (0, 0)))  # [42,42,42,48]
    win = np.lib.stride_tricks.sliding_window_view(kp, (KS, KS, KS),
                                                   axis=(0, 1, 2))
    # win: [40,40,40,48,3,3,3] -> [40,40,40,(kk,c)]
    win = np.ascontiguousarray(win.transpose(0, 1, 2, 4, 5, 6, 3))
    win = win.reshape(H, W, T, FKC)

    rpb_kh = np.ascontiguousarray(rpb.reshape(NH, NT).T).reshape(FKH)
    rpb_t = np.broadcast_to(rpb_kh, (P, FKH)).copy()

    in_maps = []
    for i in range(N_CORES):
        h0 = i * SLAB
        q_pad = np.zeros((TOKP, DIM), np.float32)
        q_pad[:TOK] = q0[h0:h0 + SLAB].reshape(TOK, DIM)
        kn_pad = np.zeros((TOKP, FKC), np.float32)
        kn_pad[:TOK] = win[h0:h0 + SLAB].reshape(TOK, FKC)
        in_maps.append({
            "qs": q_pad.reshape(TILES, P, TPP * DIM),
            "kn": kn_pad.reshape(TILES, P, TPP * FKC),
            "rpbt": rpb_t,
        })
    return in_maps


def _assemble(results):
    slabs = []
    for i in range(N_CORES):
        o = results[i]["out"].reshape(TILES, P, 3, TPP, NH)
        o = o.transpose(0, 1, 3, 2, 4).reshape(TOKP, 3, NH)[:TOK]
        o = o.reshape(SLAB, W, T, 3, NH)
        # channel order in reference: c = h*3 + o
        slabs.append(o.transpose(0, 1, 2, 4, 3).reshape(SLAB, W, T, 3 * NH))
    full = np.concatenate(slabs, axis=0)             # [40,40,40,24]
    return np.ascontiguousarray(full.transpose(3, 0, 1, 2))[None]


def _run(q, k, rpb, **spmd_kwargs):
    if "prog" not in _prog_cache:
        _prog_cache["prog"] = _build_program()
    nc = _prog_cache["prog"]
    in_maps = _host_prep(q, k, rpb)
    res = run_bass_kernel_spmd(nc, in_maps, list(range(N_CORES)),
                               **spmd_kwargs)
    return _assemble(res.results), res


def kernel(q, k, rpb):
    out, _ = _run(q, k, rpb)
    return out



# revision 13
# speedup vs baseline: 1.9735x; 1.9735x over previous
"""Trainium2 Bass kernel for 3D neighborhood attention (sparse_attention).

Problem: q,k [1,40,40,40,48] fp32, rpb [8,3,3,3]; out [1,24,40,40,40].
Per voxel x: logits[h,kk] = scale * <q[x,h,:], k[x+off_kk,h,:]> + rpb[h,kk]
(zero-padded k at boundaries, kk over 3x3x3 offsets), p = softmax over kk,
out[x,h,:] = sum_kk p[h,kk] * off_kk  (constant integer offsets as values).

Sharding: spatial-parallel over H (40 -> 8 slabs of 5), no collectives;
halo rows of k are prepared host-side.

On-core layout (the key idea): SBUF partitions = (w-column j, t-block tb)
with T split into 3 blocks of 14 (+1 halo each side inside the 16-wide
stored block), so ALL 27 neighborhood shifts are pure access-pattern
offsets: dj shifts partitions by 3*dj, di/dl shift the free offset. No
im2col, no on-chip data movement. q/k are fp16 so every DVE op runs in
2x_1P mode. Logits: 27 shifted multiplies (d-outer layout) + a binary
tree over d. rpb is a broadcast add folded before exp (exp on ScalarE,
overlapped per chunk). The PV contraction uses that values are the
constant offsets: out_o = (sum_{+1 slab} E - sum_{-1 slab} E) / sum E,
computed as fp16 slab trees reusing A[a,b] = sum_dl E partial sums.
"""

import numpy as np

import concourse.bass as bass
import concourse.tile as tile
from concourse import bacc, mybir
from concourse.bass_utils import run_bass_kernel_spmd

NH = 8
HD = 6
KS = 3
H = W = T = 40
SCALE = HD**-0.5
N_CORES = 8
SLAB = H // N_CORES          # 5 rows of H per core

TB = 3                       # t-blocks per line
TIN = 14                     # tokens per t-block (3*14 = 42 >= 40)
TQ = 16                      # stored t per block (with halo)
QP = W * TB                  # 120 partitions for q/out
KP = (W + 2) * TB            # 126 partitions for k
QF = HD * SLAB * TIN * NH    # 3360 q free size (d, i, t, h)
KF = HD * (SLAB + 2) * TQ * NH  # 5376 k free size (d, ip, tq, h)
X = SLAB * TIN * NH          # 560 = (i, t, h)
NT = KS**3                   # 27
LF = NT * X                  # 15120 logits free size (kk, i, t, h)

_prog_cache = {}


def _build_program():
    f16 = mybir.dt.float16
    f32 = mybir.dt.float32
    nc = bacc.Bacc("TRN2", target_bir_lowering=False, debug=False,
                   num_devices=N_CORES)
    qs = nc.dram_tensor("qs", [QP, QF], f16, kind="ExternalInput").ap()
    ks = nc.dram_tensor("ks", [KP, KF], f16, kind="ExternalInput").ap()
    ws = nc.dram_tensor("ws", [QP, LF], f16, kind="ExternalInput").ap()
    out = nc.dram_tensor("out", [QP, 3 * X], f32, kind="ExternalOutput").ap()

    ADD = mybir.AluOpType.add
    EXP = mybir.ActivationFunctionType.Exp

    with tile.TileContext(nc) as tc:
        with (
            tc.tile_pool(name="io", bufs=1) as iop,
            tc.tile_pool(name="work", bufs=1) as wp,
        ):
            qt = iop.tile([QP, QF], f16)
            # one k copy per dj shift (compute-engine APs must start at an
            # aligned partition, so partition-offset views are not allowed)
            kts = [iop.tile([QP, KF], f16, name=f"kt{b}") for b in range(3)]
            wt = iop.tile([QP, LF], f16)
            nc.sync.dma_start(qt[:], qs[:])
            for b in range(3):
                nc.sync.dma_start(kts[b][:], ks[3 * b:3 * b + QP])
            nc.sync.dma_start(wt[:], ws[:])

            pt = wp.tile([QP, HD * 3 * X], f16)   # products (d, a, i, t, h)
            a3 = wp.tile([QP, 3 * 3 * X], f16)    # tree stage 1 (3 planes)
            b2 = wp.tile([QP, 3 * X], f16)
            cs = wp.tile([QP, 3 * X], f16)        # logits before rpb
            lt = wp.tile([QP, LF], f16)           # logits, exp'd in place

            TH = TIN * NH                          # 112
            KTH = TQ * NH                          # 128
            qm = qt[:].rearrange("p (d i th) -> p d i th", d=HD, i=SLAB)
            kms = [kts[b][:].rearrange("p (d i th) -> p d i th",
                                       d=HD, i=SLAB + 2)
                   for b in range(3)]
            pm = pt[:].rearrange("p (d a i th) -> p d a i th",
                                 d=HD, a=3, i=SLAB)
            pd6 = pt[:].rearrange("p (d x) -> p d x", d=HD)
            a3v = a3[:].rearrange("p (d x) -> p d x", d=3)
            wv = wt[:].rearrange("p (a r x) -> p a r x", a=3, r=9)
            lv4 = lt[:].rearrange("p (a r x) -> p a r x", a=3, r=9)
            cs3 = cs[:].rearrange("p (a x) -> p a x", a=3)

            for b in range(3):
                for dl in range(3):
                    # products for the 3 kk = (a, b, dl), a in 0..2
                    for a in range(3):
                        kslice = kms[b][:, :, a:a + SLAB,
                                        dl * NH:dl * NH + TH]
                        nc.vector.tensor_mul(pm[:, :, a], qm, kslice)
                    # tree-reduce over d (outermost): 3+3 -> 3 -> 1
                    nc.vector.tensor_add(a3v[:], pd6[:, 0:3], pd6[:, 3:6])
                    nc.vector.tensor_add(b2[:], a3v[:, 0], a3v[:, 1])
                    nc.vector.tensor_add(cs[:], b2[:], a3v[:, 2])
                    # + rpb (pre-broadcast host-side) -> logits slice
                    r = 3 * b + dl
                    nc.vector.tensor_add(lv4[:, :, r], cs3[:], wv[:, :, r])
                    # exp on ScalarE (overlaps next chunk's DVE work)
                    nc.scalar.activation(lv4[:, :, r], lv4[:, :, r], EXP)

            # PV phase: values are the constant offsets in {-1,0,1}^3.
            # A[a,b] = sum_dl E; S = sum_ab A; N_a/N_b from A slabs;
            # N_l needs its own dl-slab sums.
            evk = lt[:].rearrange("p (a b dl x) -> p a b dl x", a=3, b=3, dl=3)
            a1 = wp.tile([QP, 9 * X], f16)
            aab = wp.tile([QP, 9 * X], f16)
            a1v = a1[:].rearrange("p (a b x) -> p a b x", a=3, b=3)
            aabv = aab[:].rearrange("p (a b x) -> p a b x", a=3, b=3)
            nc.vector.tensor_add(a1v[:], evk[:, :, :, 0], evk[:, :, :, 1])
            nc.vector.tensor_add(aabv[:], a1v[:], evk[:, :, :, 2])

            aab9 = aab[:].rearrange("p (n x) -> p n x", n=9)
            s4 = wp.tile([QP, 4 * X], f16)
            s2 = wp.tile([QP, 2 * X], f16)
            s1 = wp.tile([QP, X], f16)
            st = wp.tile([QP, X], f32)
            s4v = s4[:].rearrange("p (n x) -> p n x", n=4)
            s2v = s2[:].rearrange("p (n x) -> p n x", n=2)
            nc.vector.tensor_add(s4v[:], aab9[:, 0:4], aab9[:, 4:8])
            nc.vector.tensor_add(s2v[:], s4v[:, 0:2], s4v[:, 2:4])
            nc.vector.tensor_add(s1[:], s2v[:, 0], s2v[:, 1])
            nc.vector.tensor_add(st[:], s1[:], aab9[:, 8])

            nbuf = wp.tile([QP, 3 * X], f16)
            nv = nbuf[:].rearrange("p (o x) -> p o x", o=3)

            # N_a = sum_b (A[2,b] - A[0,b])
            da = wp.tile([QP, 3 * X], f16)
            dav = da[:].rearrange("p (n x) -> p n x", n=3)
            na2 = wp.tile([QP, X], f16)
            nc.vector.tensor_sub(dav[:], aabv[:, 2], aabv[:, 0])
            nc.vector.tensor_add(na2[:], dav[:, 0], dav[:, 1])
            nc.vector.tensor_add(nv[:, 0], na2[:], dav[:, 2])

            # N_b = sum_a (A[a,2] - A[a,0])
            db = wp.tile([QP, 3 * X], f16)
            dbv = db[:].rearrange("p (n x) -> p n x", n=3)
            nb2 = wp.tile([QP, X], f16)
            nc.vector.tensor_sub(dbv[:], aabv[:, :, 2], aabv[:, :, 0])
            nc.vector.tensor_add(nb2[:], dbv[:, 0], dbv[:, 1])
            nc.vector.tensor_add(nv[:, 1], nb2[:], dbv[:, 2])

            # N_l = sum_ab (E[.,.,2] - E[.,.,0])
            dlb = wp.tile([QP, 9 * X], f16)
            dl9 = dlb[:].rearrange("p (n x) -> p n x", n=9)
            dl4 = wp.tile([QP, 4 * X], f16)
            dl2 = wp.tile([QP, 2 * X], f16)
            dl1 = wp.tile([QP, X], f16)
            dl4v = dl4[:].rearrange("p (n x) -> p n x", n=4)
            dl2v = dl2[:].rearrange("p (n x) -> p n x", n=2)
            dlabv = dlb[:].rearrange("p (a b x) -> p a b x", a=3, b=3)
            nc.vector.tensor_sub(dlabv[:], evk[:, :, :, 2], evk[:, :, :, 0])
            nc.vector.tensor_add(dl4v[:], dl9[:, 0:4], dl9[:, 4:8])
            nc.vector.tensor_add(dl2v[:], dl4v[:, 0:2], dl4v[:, 2:4])
            nc.vector.tensor_add(dl1[:], dl2v[:, 0], dl2v[:, 1])
            nc.vector.tensor_add(nv[:, 2], dl1[:], dl9[:, 8])

            sinv = wp.tile([QP, X], f32)
            nc.vector.reciprocal_approx_fast(sinv[:], st[:])

            ot = wp.tile([QP, 3 * X], f32)
            ov = ot[:].rearrange("p (o x) -> p o x", o=3)
            sb = sinv[:].unsqueeze(1).broadcast_to([QP, 3, X])
            nc.vector.tensor_mul(ov[:], nv[:], sb)
            nc.sync.dma_start(out[:], ot[:])

    nc.compile()
    return nc


def _host_prep(q, k, rpb):
    q0 = (np.asarray(q, np.float32)[0] * SCALE)          # [40,40,40,48]
    k0 = np.asarray(k, np.float32)[0]
    rpb = np.asarray(rpb, np.float32)

    # padded k: [H+2, W+2, T+4, 48] (t gets 1 left + 3 right zeros so the
    # tb=2 stored block [27..43) is in range)
    kp = np.zeros((H + 2, W + 2, T + 4, NH * HD), np.float16)
    kp[1:H + 1, 1:W + 1, 1:T + 1] = k0
    # q padded in t to 42
    qp = np.zeros((H, W, TB * TIN, NH * HD), np.float16)
    qp[:, :, :T] = q0

    # rpb -> w[kk=(a,b,dl), h], pre-broadcast over (i, t) and partitions
    w_np = rpb.transpose(1, 2, 3, 0).astype(np.float16)      # [3,3,3,8]
    w_np = np.broadcast_to(w_np[:, :, :, None, None, :],
                           (3, 3, 3, SLAB, TIN, NH)).reshape(LF)
    w_rep = np.broadcast_to(w_np, (QP, LF)).copy()

    in_maps = []
    for c in range(N_CORES):
        i0 = c * SLAB
        # q_sb[j*3+tb, (d, a=1, i, t, h)] = qp[i0+i, j, tb*14+t, h*6+d]
        q_sb = np.zeros((QP, QF), np.float16)
        for tb in range(TB):
            blk = qp[i0:i0 + SLAB, :, tb * TIN:(tb + 1) * TIN, :]
            blk = blk.reshape(SLAB, W, TIN, NH, HD)
            # -> [j, d, i, t, h]
            q_sb[tb::TB] = np.ascontiguousarray(
                blk.transpose(1, 4, 0, 2, 3)).reshape(W, QF)
        # k_sb[jp*3+tb, (d, ip, tq, h)] = kp[i0+ip, jp, tb*14+tq, h*6+d]
        k_sb = np.zeros((KP, KF), np.float16)
        kc = kp[i0:i0 + SLAB + 2]
        for tb in range(TB):
            blk = kc[:, :, tb * TIN:tb * TIN + TQ, :]
            blk = blk.reshape(SLAB + 2, W + 2, TQ, NH, HD)
            k_sb[tb::TB] = np.ascontiguousarray(
                blk.transpose(1, 4, 0, 2, 3)).reshape(W + 2, KF)
        in_maps.append({"qs": q_sb, "ks": k_sb, "ws": w_rep})
    return in_maps


def _assemble(results):
    full = np.zeros((H, W, TB * TIN, NH, 3), np.float32)
    for c in range(N_CORES):
        i0 = c * SLAB
        o = results[c]["out"].reshape(W, TB, 3, SLAB, TIN, NH)
        # -> [i, j, tb, t, h, o]
        o = o.transpose(3, 0, 1, 4, 5, 2)
        full[i0:i0 + SLAB] = o.reshape(SLAB, W, TB * TIN, NH, 3)
    full = full[:, :, :T].reshape(H, W, T, NH * 3)
    return np.ascontiguousarray(full.transpose(3, 0, 1, 2))[None]


def _run(q, k, rpb, **spmd_kwargs):
    if "prog" not in _prog_cache:
        _prog_cache["prog"] = _build_program()
    nc = _prog_cache["prog"]
    in_maps = _host_prep(q, k, rpb)
    res = run_bass_kernel_spmd(nc, in_maps, list(range(N_CORES)),
                               **spmd_kwargs)
    return _assemble(res.results), res


def kernel(q, k, rpb):
    out, _ = _run(q, k, rpb)
    return out
